# revision 7
# baseline (speedup 1.0000x reference)
"""Trainium2 Bass kernel for nn_AttentionLayer (B=32, L=2048, D=1024).

reference:
    q = dh @ Wq + bq                      # [B, D]
    k = enc @ Wk + bk                     # [B, L, D]
    energy = (q . k) / sqrt(D)            # [B, L]
    energy = where(mask, -1e10, energy)
    alphas = softmax(energy, axis=1)
    context = alphas @ enc                # [B, 1, D]

Algebraic rewrite used here (exact in real arithmetic):
    energy[b,l] = enc[b,l,:] . qk_b / sqrt(D)  (+ const(b))
    qk_b = Wk @ (dh_b @ Wq + bq)
The per-row constant q.bk shifts every energy of a row equally, so softmax is
unchanged -> bk is dropped.  This turns the O(B*L*D^2) K-projection into an
O(B*D^2) matvec plus one O(B*L*D) fused multiply-reduce pass over enc, making
the kernel HBM-bound on a single read of encoder_output.

Masked rows have zero softmax weight, so their enc rows are never needed: the
host builds per-batch compacted row-index lists and the kernel gathers only
unmasked rows via indirect DMA (~50% of the traffic).

Sharding: data-parallel over batch, 4 batches per core on 8 cores; the two
1024x1024 projection weights are replicated.  No collectives.
"""

import math
import os
import sys

import numpy as np

if "/opt/trn_rl_repo" not in sys.path:
    sys.path.insert(0, "/opt/trn_rl_repo")

B, L, D = 32, 2048, 1024
NCORES = 8
BPC = B // NCORES          # batches per core
P = 128                    # partitions
DC = D // P                # 8 d-chunks of 128
SCALE = 1.0 / math.sqrt(D)

# "dense": plain DMA of all rows, mask applied as a 0/1 weight multiplier.
# "gather": indirect-DMA only the unmasked rows (host-built index lists).
MODE = os.environ.get("KERNEL_MODE", "gather")

_CACHE = {}


def _build_nc(nt, use_gather):
    """Build the per-core Bass program. nt = number of 128-row tiles per batch."""
    import concourse.bass as bass
    import concourse.bacc as bacc
    import concourse.tile as tile
    from concourse import mybir
    from concourse.masks import make_identity
    from contextlib import ExitStack

    f32 = mybir.dt.float32
    i32 = mybir.dt.int32

    nc = bacc.Bacc("TRN2", target_bir_lowering=False)

    enc = nc.dram_tensor("enc", [BPC * L, D], f32, kind="ExternalInput").ap()
    dht = nc.dram_tensor("dht", [D, BPC], f32, kind="ExternalInput").ap()
    wq = nc.dram_tensor("wq", [D, D], f32, kind="ExternalInput").ap()
    wk = nc.dram_tensor("wk", [D, D], f32, kind="ExternalInput").ap()
    bqv = nc.dram_tensor("bq", [1, D], f32, kind="ExternalInput").ap()
    keep = nc.dram_tensor("keep", [BPC, P, nt], f32, kind="ExternalInput").ap()
    if use_gather:
        gidx = nc.dram_tensor("gidx", [BPC, P, nt], i32, kind="ExternalInput").ap()
    out = nc.dram_tensor("out", [BPC, D], f32, kind="ExternalOutput").ap()

    with tile.TileContext(nc) as tc:
        with ExitStack() as ctx:
            # ---- persistent pools (live for the whole kernel) ----
            persist = ctx.enter_context(tc.tile_pool(name="persist", bufs=1))
            dram = ctx.enter_context(tc.tile_pool(name="dram", bufs=1, space="DRAM"))

            keep_sb = persist.tile([P, BPC, nt], f32)
            nc.sync.dma_start(out=keep_sb, in_=keep.rearrange("j p t -> p j t"))
            if use_gather:
                gidx_sb = persist.tile([P, BPC, nt], i32)
                nc.sync.dma_start(out=gidx_sb, in_=gidx.rearrange("j p t -> p j t"))
            ones128 = persist.tile([P, 1], f32)
            nc.vector.memset(ones128, 1.0)
            qk_dram = dram.tile([BPC, D], f32)

            # ---- setup phase: qk[b,:] = Wk @ (dh_b @ Wq + bq), scaled ----
            with (
                tc.tile_pool(name="setup", bufs=1) as setup,
                tc.tile_pool(name="wstream", bufs=3) as wstream,
                tc.tile_pool(name="setup_ps", bufs=2, space="PSUM") as setup_ps,
            ):
                ident = setup.tile([P, P], f32)
                make_identity(nc, ident)
                ones14 = setup.tile([1, BPC], f32)
                nc.vector.memset(ones14, 1.0)
                bq_sb = setup.tile([1, D], f32)
                nc.sync.dma_start(out=bq_sb, in_=bqv)
                dht_sb = setup.tile([P, DC, BPC], f32)
                nc.sync.dma_start(
                    out=dht_sb, in_=dht.rearrange("(i p) b -> p i b", p=P)
                )

                # q = dh @ Wq + bq  -> q_ps [BPC, D]
                q_ps = setup_ps.tile([BPC, D], f32, tag="acc")
                for di in range(DC):
                    wq_t = wstream.tile([P, D], f32, tag="w")
                    nc.sync.dma_start(out=wq_t, in_=wq[di * P : (di + 1) * P, :])
                    for h in range(2):
                        nc.tensor.matmul(
                            out=q_ps[:, h * 512 : (h + 1) * 512],
                            lhsT=dht_sb[:, di, :],
                            rhs=wq_t[:, h * 512 : (h + 1) * 512],
                            start=(di == 0),
                            stop=False,
                        )
                for h in range(2):
                    nc.tensor.matmul(
                        out=q_ps[:, h * 512 : (h + 1) * 512],
                        lhsT=ones14,
                        rhs=bq_sb[0:1, h * 512 : (h + 1) * 512],
                        start=False,
                        stop=True,
                    )
                q_sb = setup.tile([BPC, D], f32)
                nc.scalar.copy(q_sb, q_ps)

                # qT chunks [128e, BPC] via PE transpose
                qt_sb = setup.tile([P, DC, BPC], f32)
                for ei in range(DC):
                    qt_ps = setup_ps.tile([P, BPC], f32, tag="qt")
                    nc.tensor.transpose(
                        out=qt_ps,
                        in_=q_sb[0:BPC, ei * P : (ei + 1) * P],
                        identity=ident[0:BPC, 0:BPC],
                    )
                    nc.scalar.copy(qt_sb[:, ei, :], qt_ps)

                # WkT [e, d] via PE transpose of Wk tiles
                wkt = setup.tile([P, DC, D], f32)
                for di in range(DC):
                    wk_t = wstream.tile([P, D], f32, tag="w")
                    nc.sync.dma_start(out=wk_t, in_=wk[di * P : (di + 1) * P, :])
                    for ei in range(DC):
                        tp_ps = setup_ps.tile([P, P], f32, tag="tp")
                        nc.tensor.transpose(
                            out=tp_ps,
                            in_=wk_t[:, ei * P : (ei + 1) * P],
                            identity=ident,
                        )
                        nc.scalar.copy(wkt[:, ei, di * P : (di + 1) * P], tp_ps)

                # qk[b,d] = sum_e q[b,e] Wk[d,e]  (uses WkT as rhs)
                qk_ps = setup_ps.tile([BPC, D], f32, tag="acc")
                for ei in range(DC):
                    for h in range(2):
                        nc.tensor.matmul(
                            out=qk_ps[:, h * 512 : (h + 1) * 512],
                            lhsT=qt_sb[:, ei, :],
                            rhs=wkt[:, ei, h * 512 : (h + 1) * 512],
                            start=(ei == 0),
                            stop=(ei == DC - 1),
                        )
                qk_sb = setup.tile([BPC, D], f32)
                nc.scalar.mul(qk_sb, qk_ps, SCALE)  # fold 1/sqrt(D)
                nc.sync.dma_start(out=qk_dram, in_=qk_sb)

            # ---- main phase: per batch, energy -> softmax -> context ----
            with (
                tc.tile_pool(name="encp", bufs=nt + 6) as encp,
                tc.tile_pool(name="bcast", bufs=2) as bcast,
                tc.tile_pool(name="scratch", bufs=2) as scratchp,
                tc.tile_pool(name="small", bufs=2) as small,
                tc.tile_pool(name="main_ps", bufs=2, space="PSUM") as main_ps,
            ):
                for b in range(BPC):
                    # broadcast qk_b across all 128 partitions
                    qkb = bcast.tile([P, D], f32)
                    row = qk_dram[b : b + 1, :]
                    row_bcast = bass.AP(
                        tensor=row.tensor, offset=row.offset, ap=[[0, P], [1, D]]
                    )
                    nc.sync.dma_start(out=qkb, in_=row_bcast)

                    # load enc rows for this batch (one [128,1]-index gather
                    # per tile: HW rejects multi-column offset APs)
                    tiles = []
                    for t in range(nt):
                        e_t = encp.tile([P, D], f32, tag="enc")
                        if use_gather:
                            nc.gpsimd.indirect_dma_start(
                                out=e_t,
                                out_offset=None,
                                in_=enc,
                                in_offset=bass.IndirectOffsetOnAxis(
                                    ap=gidx_sb[:, b, t : t + 1], axis=0
                                ),
                            )
                        else:
                            r0 = b * L + t * P
                            nc.sync.dma_start(out=e_t, in_=enc[r0 : r0 + P, :])
                        tiles.append(e_t)

                    # energy[p,t] = enc_tile[p,:] . qk_b
                    # (DVE multiply, then ACT copy with free-dim accumulate)
                    ebuf = small.tile([P, nt], f32, tag="ebuf")
                    for t in range(nt):
                        scratch = scratchp.tile([P, D], f32, tag="ttr")
                        nc.vector.tensor_mul(scratch, tiles[t], qkb)
                        nc.scalar.activation(
                            out=scratch,
                            in_=scratch,
                            func=mybir.ActivationFunctionType.Copy,
                            accum_out=ebuf[:, t : t + 1],
                        )

                    # masked energies: keep_sb is 0 for valid, -1e9 for masked
                    emask = small.tile([P, nt], f32, tag="emask")
                    nc.vector.tensor_add(emask, ebuf, keep_sb[:, b, :])

                    # w = exp(e); accum gives per-partition row sums
                    wfin = small.tile([P, nt], f32, tag="wfin")
                    wsum = small.tile([P, 1], f32, tag="wsum")
                    nc.scalar.activation(
                        out=wfin,
                        in_=emask,
                        func=mybir.ActivationFunctionType.Exp,
                        accum_out=wsum,
                    )

                    # denominator = sum of weights (partition-sum via PE)
                    den_ps = main_ps.tile([1, 1], f32, tag="den")
                    nc.tensor.matmul(
                        out=den_ps, lhsT=ones128, rhs=wsum, start=True, stop=True
                    )
                    den_s = small.tile([1, 1], f32, tag="dens")
                    nc.vector.tensor_copy(den_s, den_ps)
                    rden = small.tile([1, 1], f32, tag="rden")
                    nc.vector.reciprocal(rden, den_s)

                    # context = sum_t w[:,t]^T @ enc_tile_t   (PSUM accumulate)
                    ctx_ps = [
                        main_ps.tile([1, 512], f32, tag=f"ctx{h}", name=f"ctx_ps{h}")
                        for h in range(2)
                    ]
                    for h in range(2):
                        for t in range(nt):
                            nc.tensor.matmul(
                                out=ctx_ps[h],
                                lhsT=wfin[:, t : t + 1],
                                rhs=tiles[t][:, h * 512 : (h + 1) * 512],
                                start=(t == 0),
                                stop=(t == nt - 1),
                            )
                    ctx_sb = small.tile([1, D], f32, tag="ctx")
                    for h in range(2):
                        nc.scalar.mul(
                            ctx_sb[:, h * 512 : (h + 1) * 512],
                            ctx_ps[h],
                            rden[0:1, 0:1],
                        )
                    nc.sync.dma_start(out=out[b : b + 1, :], in_=ctx_sb)

    nc.compile()
    return nc


def _prep_core_inputs(enc_np, dh_np, keepmask_np, wq_np, wk_np, bq_np, nt, use_gather):
    """Build the 8 per-core input maps. keepmask_np: True where attendable."""
    in_maps = []
    for c in range(NCORES):
        b0 = c * BPC
        m = {
            "enc": np.ascontiguousarray(
                enc_np[b0 : b0 + BPC].reshape(BPC * L, D)
            ),
            "dht": np.ascontiguousarray(dh_np[b0 : b0 + BPC].T),
            "wq": wq_np,
            "wk": wk_np,
            "bq": bq_np.reshape(1, D),
        }
        # additive mask: 0.0 where attended, -1e9 where masked/padded
        keep = np.zeros((BPC, P, nt), np.float32)
        if use_gather:
            gidx = np.zeros((BPC, P, nt), np.int32)
        for j in range(BPC):
            km = keepmask_np[b0 + j]
            if use_gather:
                rows = np.flatnonzero(km).astype(np.int32)
                n = len(rows)
                arr = np.full(nt * P, rows[0], np.int32)
                arr[:n] = rows
                valid = np.full(nt * P, -1e9, np.float32)
                valid[:n] = 0.0
                gidx[j] = (arr + j * L).reshape(nt, P).T
                keep[j] = valid.reshape(nt, P).T
            else:
                keep[j] = np.where(km, 0.0, -1e9).astype(np.float32).reshape(nt, P).T
        m["keep"] = keep
        if use_gather:
            m["gidx"] = gidx
        in_maps.append(m)
    return in_maps


def kernel(
    encoder_output,
    decoder_hidden_state,
    mask,
    max_src_length=None,
    Wq=None,
    bq=None,
    Wk=None,
    bk=None,
    **_unused,
):
    from concourse.bass_utils import run_bass_kernel_spmd

    enc_np = np.asarray(encoder_output, np.float32)
    dh_np = np.asarray(decoder_hidden_state, np.float32)
    mask_np = np.asarray(mask, bool)
    wq_np = np.ascontiguousarray(np.asarray(Wq, np.float32))
    wk_np = np.ascontiguousarray(np.asarray(Wk, np.float32))
    bq_np = np.asarray(bq, np.float32)
    # bk is intentionally unused: q.bk is constant per row -> softmax invariant.

    keepmask = ~mask_np  # True where the position is attended
    use_gather = MODE == "gather"
    if use_gather:
        max_keep = int(keepmask.sum(axis=1).max())
        nt = max(1, math.ceil(max_keep / P))
    else:
        nt = L // P

    key = (nt, use_gather)
    if key not in _CACHE:
        _CACHE[key] = _build_nc(nt, use_gather)
    nc = _CACHE[key]

    in_maps = _prep_core_inputs(
        enc_np, dh_np, keepmask, wq_np, wk_np, bq_np, nt, use_gather
    )
    res = run_bass_kernel_spmd(nc, in_maps, core_ids=list(range(NCORES)))
    out = np.concatenate([res.results[c]["out"] for c in range(NCORES)], axis=0)
    return out.reshape(B, 1, D).astype(np.float32)


if __name__ == "__main__":
    sys.path.insert(0, os.path.dirname(os.path.abspath(__file__)))
    import reference

    inputs = reference.setup_inputs()
    expected = np.asarray(reference.reference(**inputs))
    actual = kernel(**{k: np.asarray(v) for k, v in inputs.items()})
    err = np.abs(actual - expected).max() / max(np.abs(expected).max(), 1e-30)
    print("Relative error:", err)
